# revision 1
# baseline (speedup 1.0000x reference)
"""3-layer GCN (PyG GCNConv x3) on 8 Trainium2 NeuronCores.

Strategy (sharding_hint: partition edges by destination node):
  - Each core owns N/8 destination nodes and the edges pointing at them.
  - Reformulation: per layer  g = (h @ W) * dinv ;  s[d] = sum_{e: dst=d} g[src[e]] ;
    out = (s + g) * dinv + b.  All three layers (even D_out=1) share one
    64-wide gather + segment-sum engine (layer 3 applies W3 after aggregation).
  - Gather of g[src] uses dma_gather (SWDGE) over 4 queues; src space is split
    into 4 chunks of N/4 so indices fit int16.
  - Segment-sum runs on the TensorEngine as one-hot "staircase" matmuls with
    R matrices generated on-device (DVE is_equal vs an iota ramp).  Edges are
    host-sorted by (src_chunk, dst_block) and each (chunk, dst-block) cell is
    padded to a fixed capacity C so the instruction schedule is identical on
    all 8 cores (SPMD).
  - Layer 1 computes g for ALL nodes redundantly on every core from a
    replicated x (no collective); layers 2/3 AllGather the per-shard g.

kernel() takes full inputs, does index/sort preprocessing on host, runs the
bass kernel on cores 0-7, and returns the full [N, 1] output.
"""

import dataclasses
import numpy as np

import concourse.bass as bass
import concourse.tile as tile
from concourse import bacc, mybir
from concourse.library_config import mlp as _mlp_lib
from concourse.masks import make_identity
from concourse.bass_utils import run_bass_kernel_spmd

NCORES = 8
NCHUNK = 4
P = 128
D = 64
TG = 2048              # gather-call size (slots)
PAD_DREL = 200.0       # sentinel rel-dst for padded slots (no iota match)


def _cdiv(a, b):
    return (a + b - 1) // b


def _bcast_inner(ap, n):
    """[.., k] AP -> [.., k, n] with a stride-0 inner broadcast dim."""
    return dataclasses.replace(ap, ap=list(ap.ap) + [[0, n]])


def _host_prep(x, edge_index):
    """Shard + sort edges, build slot streams and packed operands."""
    N = x.shape[0]
    assert N % NCORES == 0 and (N // 2) % 1 == 0
    SH = N // NCORES
    NBLK = _cdiv(SH, P)
    CH = N // NCHUNK
    assert CH <= 32767, "int16 gather index limit"

    src = np.asarray(edge_index[0], dtype=np.int64)
    dst = np.asarray(edge_index[1], dtype=np.int64)

    deg = np.bincount(dst, minlength=N).astype(np.float64) + 1.0
    dinv = (1.0 / np.sqrt(deg)).astype(np.float32)

    per_core = []
    maxcell = 1
    for c in range(NCORES):
        sel = (dst >= c * SH) & (dst < (c + 1) * SH)
        es = src[sel]
        ed = dst[sel] - c * SH
        ch = es // CH
        bl = ed // P
        cell = ch * NBLK + bl
        order = np.argsort(cell, kind="stable")
        es, ed, ch, bl, cell = es[order], ed[order], ch[order], bl[order], cell[order]
        counts = np.bincount(cell, minlength=NCHUNK * NBLK)
        maxcell = max(maxcell, int(counts.max()))
        per_core.append((es, ed, ch, bl, cell, counts))

    C = max(P, _cdiv(maxcell, P) * P)
    TOT = NCHUNK * NBLK * C

    idxw_l, drel_l = [], []
    for es, ed, ch, bl, cell, counts in per_core:
        starts = np.zeros(NCHUNK * NBLK, np.int64)
        starts[1:] = np.cumsum(counts)[:-1]
        pos = np.arange(es.shape[0]) - np.repeat(starts, counts)
        slot = cell * C + pos
        gidx = np.zeros(TOT, np.int16)
        drel = np.full(TOT, PAD_DREL, np.float32)
        gidx[slot] = (es - ch * CH).astype(np.int16)
        drel[slot] = (ed - bl * P).astype(np.float32)
        idxw = np.tile(gidx.reshape(TOT // 16, 16).T, (8, 1)).copy()   # [128, TOT/16]
        drw = drel.reshape(TOT // P, P).T.copy()                       # [128, TOT/128]
        idxw_l.append(idxw)
        drel_l.append(drw)

    return SH, NBLK, CH, C, TOT, dinv, idxw_l, drel_l


def _build(N, SH, NBLK, C, TOT, bias1_zero, bias2_zero, b3_val):
    """Build the SPMD bass program (identical on all cores)."""
    HALF = N // 2
    HT = _cdiv(HALF, P)          # matmul tiles per half (xT2 packing)
    CP = C // P                  # staircase passes per cell
    S = NBLK * C                 # slots per chunk
    f32 = mybir.dt.float32
    bf16 = mybir.dt.float16

    nc = bacc.Bacc("TRN2", target_bir_lowering=False, debug=False,
                   num_devices=NCORES, num_swdge_queues=4)

    # ---- I/O ----
    xT2 = nc.dram_tensor("xT2", [P, HALF], f32, kind="ExternalInput")
    xTsh = nc.dram_tensor("xTsh", [D, SH], f32, kind="ExternalInput")
    idxw = nc.dram_tensor("idxw", [P, TOT // 16], mybir.dt.int16, kind="ExternalInput")
    dreld = nc.dram_tensor("dreld", [P, TOT // P], f32, kind="ExternalInput")
    dinv_tiles = nc.dram_tensor("dinv_tiles", [P, 2 * HT], f32, kind="ExternalInput")
    dinv_shard = nc.dram_tensor("dinv_shard", [P, NBLK], f32, kind="ExternalInput")
    w1 = nc.dram_tensor("Wrep1", [P, D], f32, kind="ExternalInput")
    w2 = nc.dram_tensor("Wrep2", [P, D], f32, kind="ExternalInput")
    w3 = nc.dram_tensor("W3rep", [P, D], f32, kind="ExternalInput")
    b1d = nc.dram_tensor("b1rep", [P, D], f32, kind="ExternalInput")
    b2d = nc.dram_tensor("b2rep", [P, D], f32, kind="ExternalInput")
    iotad = nc.dram_tensor("iota", [P, 16 * P], f32, kind="ExternalInput")
    outd = nc.dram_tensor("out", [SH, 1], f32, kind="ExternalOutput")

    gA = nc.dram_tensor("gA", [N, D], f32)                       # layer-1 g (local full)
    gsh_dram = nc.dram_tensor("gsh_dram", [SH, D], f32)          # shard g for AG
    ush_dram = nc.dram_tensor("ush_dram", [SH, D], f32)
    gB = nc.dram_tensor("gB", [N, D], f32, addr_space="Shared")  # AG outputs
    gC = nc.dram_tensor("gC", [N, D], f32, addr_space="Shared")

    from contextlib import ExitStack
    from concourse.tile import add_dep_helper
    with tile.TileContext(nc) as tc, ExitStack() as ctx:
        libload = nc.gpsimd.load_library(_mlp_lib)
        cpool = ctx.enter_context(tc.tile_pool(name="consts", bufs=1))
        xpool = ctx.enter_context(tc.tile_pool(name="xbuf", bufs=2))
        gstpool = ctx.enter_context(tc.tile_pool(name="gstage", bufs=2))
        ipool = ctx.enter_context(tc.tile_pool(name="idxt", bufs=8))
        mpool = ctx.enter_context(tc.tile_pool(name="msgs", bufs=8))
        mbpool = ctx.enter_context(tc.tile_pool(name="msgsb", bufs=6))
        rpool = ctx.enter_context(tc.tile_pool(name="rgen", bufs=3))
        bigpool = ctx.enter_context(tc.tile_pool(name="big", bufs=1))
        epool = ctx.enter_context(tc.tile_pool(name="epi", bufs=8))
        pp_cell = ctx.enter_context(tc.tile_pool(name="pcell", bufs=4, space="PSUM"))
        pp_g = ctx.enter_context(tc.tile_pool(name="pg", bufs=2, space="PSUM"))
        pp_t = ctx.enter_context(tc.tile_pool(name="pt", bufs=2, space="PSUM"))

        # ---- constants in SBUF ----
        def cload(dram, shape, tag):
            t = cpool.tile(shape, f32, tag=tag)
            nc.sync.dma_start(t[:], dram[:])
            return t

        w1_sb = cload(w1, [P, D], "w1c")
        w2_sb = cload(w2, [P, D], "w2c")
        w3_sb = cload(w3, [P, D], "w3c")
        b1_sb = None if bias1_zero else cload(b1d, [P, D], "b1c")
        b2_sb = None if bias2_zero else cload(b2d, [P, D], "b2c")
        dit_sb = cload(dinv_tiles, [P, 2 * HT], "ditc")
        dis_sb = cload(dinv_shard, [P, NBLK], "disc")
        iota_sb = cload(iotad, [P, 16 * P], "iotac")
        ident = cpool.tile([P, P], f32)
        make_identity(nc, ident[:])

        acc = bigpool.tile([P, NBLK * D], f32, tag="acc")
        gsh = bigpool.tile([P, NBLK * D], f32, tag="gsh")
        hbuf = bigpool.tile([P, NBLK * D], f32, tag="hbuf")
        ostage = bigpool.tile([P, NBLK], f32, tag="ostage")

        # ================= layer-1 g over ALL nodes (replicated x) ==========
        XB = 1024                 # xT2 cols per load
        nxb = _cdiv(HALF, XB)
        for xb in range(nxb):
            c0 = xb * XB
            cn = min(XB, HALF - c0)
            xt = xpool.tile([P, XB], f32, tag="xb")
            nc.sync.dma_start(xt[:, :cn], xT2[:, c0:c0 + cn])
            nts = _cdiv(cn, P)
            for half in (0, 1):
                bp = half * D
                st = gstpool.tile([P, 8, D], f32, tag="gst")
                for j in range(nts):
                    ctn = min(P, cn - j * P)
                    ps = pp_g.tile([P, D], f32)
                    nc.tensor.matmul(
                        out=ps[:ctn, :],
                        lhsT=xt[bp:bp + D, j * P:j * P + ctn],
                        rhs=w1_sb[bp:bp + D, :],
                        start=True, stop=True)
                    tcol = half * HT + xb * 8 + j
                    nc.vector.tensor_scalar_mul(
                        st[:ctn, j, :], ps[:ctn, :], dit_sb[:ctn, tcol:tcol + 1])
                # flush: nodes [half*HALF + c0, +cn)
                r0 = half * HALF + c0
                dview = gA[r0:r0 + cn, :].rearrange("(t p) d -> p t d", p=P) \
                    if cn % P == 0 else None
                if dview is not None:
                    nc.sync.dma_start(dview, st[:, :nts, :])
                else:
                    full = cn // P
                    if full:
                        nc.sync.dma_start(
                            gA[r0:r0 + full * P, :].rearrange("(t p) d -> p t d", p=P),
                            st[:, :full, :])
                    rem = cn - full * P
                    nc.sync.dma_start(gA[r0 + full * P:r0 + cn, :], st[:rem, full, :])

        # ---- layer-1 g for own shard (self-loop term), kept in SBUF -------
        nshb = _cdiv(SH, XB)
        for xb in range(nshb):
            c0 = xb * XB
            cn = min(XB, SH - c0)
            xt = xpool.tile([D, XB], f32, tag="xsh")
            nc.sync.dma_start(xt[:D, :cn], xTsh[:, c0:c0 + cn])
            for j in range(_cdiv(cn, P)):
                b = xb * 8 + j
                ctn = min(P, cn - j * P)
                ps = pp_g.tile([P, D], f32)
                nc.tensor.matmul(out=ps[:ctn, :], lhsT=xt[:D, j * P:j * P + ctn],
                                 rhs=w1_sb[:D, :], start=True, stop=True)
                nc.vector.tensor_scalar_mul(
                    gsh[:ctn, b * D:(b + 1) * D], ps[:ctn, :], dis_sb[:ctn, b:b + 1])

        # ================= aggregation engine ==============================
        def agg(gdram, CHn):
            callno = 0
            open_ps = [None]
            for ch in range(NCHUNK):
                gsrc = gdram[ch * CHn:(ch + 1) * CHn, :]
                done = 0
                while done < S:
                    tg = min(TG, S - done)
                    npass = tg // P
                    base = ch * S + done
                    it = ipool.tile([P, TG // 16], mybir.dt.int16, tag="it")
                    nc.sync.dma_start(it[:, :tg // 16],
                                      idxw[:, base // 16:(base + tg) // 16])
                    mt = mpool.tile([P, TG // P, D], f32, tag="mt")
                    gi = nc.gpsimd.dma_gather(
                        out_ap=mt[:, :npass, :], in_ap=gsrc, idxs_ap=it[:, :tg // 16],
                        num_idxs=tg, num_idxs_reg=tg, elem_size=D,
                        single_packet=False, queue_num=callno % 4)
                    add_dep_helper(gi.ins, libload.ins, True, "lib before gather")
                    mtb = mbpool.tile([P, TG // P, D], bf16, tag="mtb")
                    nc.scalar.copy(mtb[:, :npass, :], mt[:, :npass, :])
                    rt = rpool.tile([P, TG // P, P], bf16, tag="rt")
                    c0 = base // P
                    drt = ipool.tile([P, TG // P], f32, tag="drt")
                    nc.sync.dma_start(drt[:, :npass], dreld[:, c0:c0 + npass])
                    nc.vector.tensor_tensor(
                        out=rt[:, :npass, :],
                        in0=iota_sb[:].rearrange("p (a b) -> p a b", b=P)[:, :npass, :],
                        in1=_bcast_inner(drt[:, :npass], P),
                        op=mybir.AluOpType.is_equal)
                    for p_i in range(npass):
                        slot0 = base + p_i * P
                        cell = slot0 // C
                        pos = (slot0 % C) // P
                        blkid = cell % NBLK
                        if pos == 0:
                            cellps = pp_cell.tile([P, D], f32, tag="cellps")
                            open_ps[0] = cellps
                        nc.tensor.matmul(
                            out=open_ps[0][:], lhsT=rt[:, p_i, :], rhs=mtb[:, p_i, :],
                            start=(pos == 0), stop=(pos == CP - 1))
                        if pos == CP - 1:
                            dstsl = acc[:, blkid * D:(blkid + 1) * D]
                            if ch == 0:
                                nc.vector.tensor_copy(dstsl, open_ps[0][:])
                            else:
                                nc.vector.tensor_add(dstsl, dstsl, open_ps[0][:])
                    done += tg
                    callno += 1

        # ================= epilogues =======================================
        def epi12(layer, b_sb):
            """out=(acc+gsh)*dinv+b, relu -> hbuf; layer2: also u."""
            stu = None
            for b in range(NBLK):
                sl = slice(b * D, (b + 1) * D)
                t1 = epool.tile([P, D], f32, tag="t1")
                nc.vector.tensor_add(t1[:], acc[:, sl], gsh[:, sl])
                h = hbuf[:, sl]
                if b_sb is None:
                    nc.vector.tensor_scalar(
                        out=h, in0=t1[:], scalar1=dis_sb[:, b:b + 1],
                        scalar2=0.0, op0=mybir.AluOpType.mult,
                        op1=mybir.AluOpType.max)
                else:
                    t2 = epool.tile([P, D], f32, tag="t2")
                    nc.vector.scalar_tensor_tensor(
                        out=t2[:], in0=t1[:], scalar=dis_sb[:, b:b + 1],
                        in1=b_sb[:], op0=mybir.AluOpType.mult,
                        op1=mybir.AluOpType.add)
                    nc.vector.tensor_scalar_max(h, t2[:], 0.0)
                if layer == 2:
                    # u = h * dinv -> gsh slot + staged DRAM write
                    u = epool.tile([P, D], f32, tag="u")
                    nc.vector.tensor_scalar_mul(u[:], h, dis_sb[:, b:b + 1])
                    nc.vector.tensor_copy(gsh[:, sl], u[:])
                    if b % 16 == 0:
                        stu = gstpool.tile([P, 16, D], f32, tag="ust")
                    nc.vector.tensor_copy(stu[:, b % 16, :], u[:])
                    if b % 16 == 15 or b == NBLK - 1:
                        nb = b % 16 + 1
                        r0 = (b - nb + 1) * P
                        cn = min(nb * P, SH - r0)
                        full = cn // P
                        if full:
                            nc.sync.dma_start(
                                ush_dram[r0:r0 + full * P, :]
                                .rearrange("(t p) d -> p t d", p=P),
                                stu[:, :full, :])
                        if cn - full * P:
                            nc.sync.dma_start(ush_dram[r0 + full * P:r0 + cn, :],
                                              stu[:cn - full * P, full, :])

        # ================= run the three layers ============================
        CHn = N // NCHUNK
        agg(gA, CHn)          # layer-1 aggregation
        epi12(1, b1_sb)

        # layer-2 g-matmul: per-tile transpose of hbuf, then matmul
        stg = None
        for b in range(NBLK):
            nt = min(P, SH - b * P)
            pt = pp_t.tile([P, P], f32, tag="pt")
            nc.tensor.transpose(pt[:D, :nt], hbuf[:nt, b * D:(b + 1) * D],
                                ident[:nt, :nt])
            hT = epool.tile([D, P], f32, tag="hTt")
            nc.vector.tensor_copy(hT[:, :nt], pt[:D, :nt])
            ps = pp_g.tile([P, D], f32)
            nc.tensor.matmul(
                out=ps[:nt, :], lhsT=hT[:, :nt],
                rhs=w2_sb[:D, :], start=True, stop=True)
            g2 = epool.tile([P, D], f32, tag="g2")
            nc.vector.tensor_scalar_mul(g2[:nt, :], ps[:nt, :], dis_sb[:nt, b:b + 1])
            nc.vector.tensor_copy(gsh[:, b * D:(b + 1) * D], g2[:])
            if b % 16 == 0:
                stg = gstpool.tile([P, 16, D], f32, tag="g2st")
            nc.vector.tensor_copy(stg[:, b % 16, :], g2[:])
            if b % 16 == 15 or b == NBLK - 1:
                nb = b % 16 + 1
                r0 = (b - nb + 1) * P
                cn = min(nb * P, SH - r0)
                full = cn // P
                if full:
                    nc.sync.dma_start(
                        gsh_dram[r0:r0 + full * P, :].rearrange("(t p) d -> p t d", p=P),
                        stg[:, :full, :])
                if cn - full * P:
                    nc.sync.dma_start(gsh_dram[r0 + full * P:r0 + cn, :],
                                      stg[:cn - full * P, full, :])

        nc.gpsimd.collective_compute(
            "AllGather", mybir.AluOpType.bypass,
            replica_groups=[list(range(NCORES))],
            ins=[gsh_dram[:]], outs=[gB[:]])
        agg(gB, CHn)
        epi12(2, b2_sb)

        nc.gpsimd.collective_compute(
            "AllGather", mybir.AluOpType.bypass,
            replica_groups=[list(range(NCORES))],
            ins=[ush_dram[:]], outs=[gC[:]])
        agg(gC, CHn)

        # layer-3 epilogue: out = ((acc+u)*dinv) @ W3 + b3
        for b in range(NBLK):
            sl = slice(b * D, (b + 1) * D)
            t1 = epool.tile([P, D], f32, tag="t1")
            nc.vector.tensor_add(t1[:], acc[:, sl], gsh[:, sl])
            t3 = epool.tile([P, D], f32, tag="t3")
            nc.vector.scalar_tensor_tensor(
                out=t3[:], in0=t1[:], scalar=dis_sb[:, b:b + 1], in1=w3_sb[:],
                op0=mybir.AluOpType.mult, op1=mybir.AluOpType.mult,
                accum_out=ostage[:, b:b + 1])
        if b3_val != 0.0:
            nc.vector.tensor_scalar_add(ostage[:], ostage[:], float(b3_val))
        nfull = SH // P
        nc.sync.dma_start(
            outd[:nfull * P, :].rearrange("(b p) o -> p (b o)", p=P),
            ostage[:, :nfull])
        if SH - nfull * P:
            nc.sync.dma_start(outd[nfull * P:, :],
                              ostage[:SH - nfull * P, nfull:nfull + 1])

    nc.compile()
    return nc


_CACHE = {}


def kernel(x, edge_index, W1, b1, W2, b2, W3, b3, _trace=False):
    x = np.asarray(x, np.float32)
    N = x.shape[0]
    SH, NBLK, CH, C, TOT, dinv, idxw_l, drel_l = _host_prep(x, edge_index)
    HALF = N // 2
    HT = _cdiv(HALF, P)

    b1 = np.asarray(b1, np.float32); b2 = np.asarray(b2, np.float32)
    b3 = np.asarray(b3, np.float32)
    W1 = np.asarray(W1, np.float32); W2 = np.asarray(W2, np.float32)
    W3 = np.asarray(W3, np.float32)
    b1z = bool(np.all(b1 == 0)); b2z = bool(np.all(b2 == 0))

    key = (N, SH, C, b1z, b2z, float(b3[0]))
    if key not in _CACHE:
        _CACHE[key] = _build(N, SH, NBLK, C, TOT, b1z, b2z, float(b3[0]))
    nc = _CACHE[key]

    xT2 = np.concatenate([x[:HALF].T, x[HALF:].T], axis=0).copy()    # [128, HALF]
    dint = np.zeros((P, 2 * HT), np.float32)
    for half in (0, 1):
        for j in range(HT):
            s0 = half * HALF + j * P
            nt = min(P, HALF - j * P)
            dint[:nt, half * HT + j] = dinv[s0:s0 + nt]
    iota = np.tile(np.arange(P, dtype=np.float32), (P, 16)).reshape(P, 16 * P)
    w1r = np.concatenate([W1, W1], axis=0)
    w2r = np.concatenate([W2, W2], axis=0)
    w3r = np.tile(W3[:, 0], (P, 1))
    b1r = np.tile(b1, (P, 1)); b2r = np.tile(b2, (P, 1))

    in_maps = []
    for c in range(NCORES):
        dis = np.zeros((P, NBLK), np.float32)
        for b in range(NBLK):
            s0 = c * SH + b * P
            nt = min(P, SH - b * P)
            dis[:nt, b] = dinv[s0:s0 + nt]
        in_maps.append({
            "xT2": xT2, "xTsh": x[c * SH:(c + 1) * SH].T.copy(),
            "idxw": idxw_l[c], "dreld": drel_l[c],
            "dinv_tiles": dint, "dinv_shard": dis,
            "Wrep1": w1r, "Wrep2": w2r, "W3rep": w3r,
            "b1rep": b1r, "b2rep": b2r, "iota": iota,
        })

    res = run_bass_kernel_spmd(nc, in_maps, core_ids=list(range(NCORES)),
                               trace=_trace)
    out = np.concatenate([res.results[c]["out"] for c in range(NCORES)], axis=0)
    if _trace:
        return out, res
    return out

